# revision 1
# baseline (speedup 1.0000x reference)
"""DTW similarity kernel for Trainium2 (8 NeuronCores, batch-sharded).

Reference computation (per batch):
  C[i,j] = ||seq1[i]-seq2[j]||_2   (1024x1024, via GEMM)
  D[i,j] = C[i-1,j-1] + min(D[i-1,j-1], D[i-1,j], D[i,j-1])  (DTW DP)
  out    = softmax(scale_weights)[0] / (1 + D[L,L]/(2L))

Device algorithm (per core, B_loc=4 batches), partitions p = jb*B_loc + b
(jb = column-block of WB=32 DP columns, NB=32 blocks):

  Phase 1  GEMM (transposed): psum[j,i] = (-2*seq2^T)^T @ seq1^T + K=2
           accumulate of sq2[j]+sq1[i]  ->  ACT sqrt  ->  stage = C^T tile.
  Phase 2  Relayout: SBUF->SBUF DMA with 2KB-contiguous runs into per-
           partition strips laid out [c-lane][slot], slot = i + SK*jb
           (skewed), unfilled slots zeroed.
  Phase 3  Row-wavefront scan, step t: partition (jb,b) computes DP row
           i = t-2*jb of its block with ONE tensor_tensor_scan
           (state = min(m[j], state) + C[j]) and ONE tensor_tensor min
           (m).  Cross-partition handoff: GPSIMD pre-min q = min(cur,
           prev) last cols, PE shift-matmul into PSUM (+BIG for jb=0),
           scan reads it directly as `initial` SK steps later.
"""

import contextlib

import numpy as np

import concourse.bacc as bacc
import concourse.bass as bass
import concourse.mybir as mybir
from concourse import bass_utils
from concourse.mybir import AluOpType
from concourse.tile import TileContext

F32 = mybir.dt.float32
BIG = 1e9

B_FULL, L_FULL, F_FULL = 32, 1024, 128
N_CORES = 8


def build_program(B_loc=4, L=1024, Fdim=128, NB=32, WB=32, phase="all"):
    """Build the Bass program for one core handling B_loc batches."""
    assert NB * WB == L and B_loc * NB <= 128 and Fdim <= 128
    P = B_loc * NB
    SK = 2                    # wavefront skew: t = i + SK*jb
    NSTEPS = L + SK * (NB - 1)
    SLOTS = NSTEPS
    W = WB * SLOTS            # per-partition strip width (elements)
    NJB = L // 128            # 128-wide j-blocks (GEMM lhsT tiles)
    JPT = 128 // WB           # jb column-blocks per GEMM tile
    IW = min(L, 512)          # i-chunk (GEMM N)
    NIH = L // IW
    assert L % 128 == 0 and (L % 512 == 0 or L < 512)

    nc = bacc.Bacc(
        "TRN2", debug=False, num_devices=N_CORES,
        # The race detector cannot decompose the mixed partition+offset
        # steps of the relayout DMA dest AP and reports phantom races
        # between disjoint tensors; Tile's dependency tracking handles
        # them (verified via emitted waits + end-to-end checks).
        detect_race_conditions=False,
    )

    # ---- I/O ----
    # s12: cols [0,L) = -2*seq2^T (lhsT), cols [L,2L) = seq1^T (rhs)
    s12_d = nc.dram_tensor("s12t", (Fdim, B_loc * 2 * L), F32, kind="ExternalInput")
    # sq1 rows, per batch concatenated
    sq1_d = nc.dram_tensor("sq1r", (1, B_loc * L), F32, kind="ExternalInput")
    # packed constants: cols [0,128) shift matrix; [128,128+B_loc) bigfix
    # row (partition 0); col 256 ones; col 257 zcol; col 258 bigcol;
    # cols [259, 259+B_loc*NJB) sq2 columns; [259+B_loc*NJB, +128) ones row
    NCST = 259 + B_loc * NJB + 128
    cst_d = nc.dram_tensor("cpack", (128, NCST), F32, kind="ExternalInput")
    out_d = nc.dram_tensor("simout", (B_loc,), F32, kind="ExternalOutput")

    with TileContext(nc) as tc:
        with contextlib.ExitStack() as ctx:
            const_pool = ctx.enter_context(tc.tile_pool(name="const", bufs=1))
            seq_pool = ctx.enter_context(tc.tile_pool(name="seqs", bufs=1))
            stage_pool = ctx.enter_context(tc.tile_pool(name="stage", bufs=3))
            strip_pool = ctx.enter_context(tc.tile_pool(name="strip", bufs=1))
            r_pool = ctx.enter_context(tc.tile_pool(name="rtiles", bufs=1))
            m_pool = ctx.enter_context(tc.tile_pool(name="mtiles", bufs=3))
            q_pool = ctx.enter_context(tc.tile_pool(name="qtiles", bufs=1))
            fin_pool = ctx.enter_context(tc.tile_pool(name="fin", bufs=1))
            ps_mm = ctx.enter_context(tc.tile_pool(name="psmm", bufs=3, space="PSUM"))
            ps_col = ctx.enter_context(tc.tile_pool(name="pscol", bufs=5, space="PSUM"))

            # ---- constants / inputs to SBUF (single DMA each) ----
            cst = const_pool.tile([128, NCST], F32)
            nc.sync.dma_start(cst[:], cst_d[:, :])
            sh_sb = cst[0:P, 0:P]             # shift matrix lhsT
            bf_sb = cst[0:1, 128:128 + P]  # bigfix lhsT row (BIG, then 0s)
            ones1 = cst[0:1, 256:257]
            zcol = cst[0:P, 257:258]           # scan_0 initial
            bigcol = cst[0:P, 258:259]         # scan_1/2 initial
            onesrow = cst[0:1, 259 + B_loc * NJB:259 + B_loc * NJB + 128]

            s12_sb = seq_pool.tile([Fdim, B_loc * 2 * L], F32)
            nc.sync.dma_start(s12_sb[:], s12_d[:, :])
            sq1_sb = seq_pool.tile([1, B_loc * L], F32)
            nc.sync.dma_start(sq1_sb[:], sq1_d[:, :])

            # C strips (zeroed guards), flat layout c*SLOTS + slot
            strip = strip_pool.tile([P, W], F32)
            nc.gpsimd.memset(strip[:], 0.0)
            stripv = strip[:].rearrange("p (c s) -> p c s", c=WB)

            # R tiles all-BIG; halo col 0 stays BIG forever.  6-deep
            # rotation keeps every cross-engine WAR several steps slack.
            NR = 6
            R = [r_pool.tile([P, WB + 1], F32, tag=f"R{k}", name=f"R{k}")
                 for k in range(NR)]
            for k in range(NR):
                nc.gpsimd.memset(R[k][:], BIG)

            # q tiles (full 128 partitions: GPSIMD needs 16-aligned spans)
            NQ = 5
            qts = [q_pool.tile([P, 1], F32, tag=f"qt{k}", name=f"qt{k}")
                   for k in range(NQ)]

            # PE warm-up: observe setup writers before the loops (uses a
            # pcol-pool slot so no extra PSUM bank is consumed)
            wps = ps_col.tile([P, 1], F32, tag="pcol", name="wps")
            nc.tensor.matmul(wps[:], sh_sb, zcol, start=True, stop=False)
            for k in range(NR):
                nc.tensor.matmul(
                    wps[:], sh_sb, R[k][:, 0:1],
                    start=False, stop=(k == NR - 1),
                )

            # ---- GEMM (C^T) + sqrt + relayout-to-strips ----
            if phase in ("all", "gemm"):
                for ih in range(NIH):
                    for jB in range(NJB):
                        for b in range(B_loc):
                            o = b * 2 * L
                            pt = ps_mm.tile([128, IW], F32, tag="pmm", name="pmm")
                            nc.tensor.matmul(
                                pt[:],
                                s12_sb[:, o + jB * 128:o + (jB + 1) * 128],
                                s12_sb[:, o + L + ih * IW:o + L + (ih + 1) * IW],
                                start=True, stop=False,
                            )
                            nc.tensor.matmul(
                                pt[:],
                                onesrow,
                                sq1_sb[0:1, b * L + ih * IW:b * L + (ih + 1) * IW],
                                start=False, stop=True,
                            )
                            st = stage_pool.tile([128, IW], F32, tag="cstage",
                                                 name="cstage")
                            # sqrt with per-partition (j) bias = sq2[j]
                            sq2col = cst[0:128, 259 + b * NJB + jB:
                                         260 + b * NJB + jB]
                            nc.scalar.activation(
                                st[:], pt[:], mybir.ActivationFunctionType.Sqrt,
                                bias=sq2col, scale=1.0,
                            )
                            # relayout: stage (128 j, IW i) -> strips,
                            # one DMA per jb (partition-contained views)
                            jb0 = jB * JPT
                            for u in range(JPT):
                                jb = jb0 + u
                                p = jb * B_loc + b
                                o = ih * IW + SK * jb
                                dst = stripv[p:p + 1, :, o:o + IW]
                                eng = (nc.sync, nc.scalar, nc.gpsimd)[
                                    (b * JPT + u) % 3]
                                eng.dma_start(dst, st[u * WB:(u + 1) * WB, :])

            # ---- scan phase ----
            if phase in ("all", "scan"):
                pcols = [None] * SK  # pcol_{t-SK} ... pcol_{t-1}
                for t in range(NSTEPS):
                    cur, prev = t % NR, (t - 1) % NR
                    # m_t
                    mt = m_pool.tile([P, WB], F32, tag="m", name="m")
                    nc.vector.tensor_tensor(
                        mt[:], R[prev][:, 0:WB], R[prev][:, 1:WB + 1],
                        AluOpType.min,
                    )
                    # scan_t: data1 = strip column t (stride SLOTS over c)
                    if t == 0:
                        initial = zcol
                    elif t < SK:
                        initial = bigcol
                    else:
                        initial = pcols[0][:]
                    data1 = stripv[:, :, t]
                    nc.vector.tensor_tensor_scan(
                        R[cur][:, 1:WB + 1],
                        mt[:],
                        data1,
                        initial,
                        AluOpType.min,
                        AluOpType.add,
                    )
                    # Pool: q = min(lastcol_t, lastcol_{t-1})
                    qt = qts[t % NQ]
                    nc.gpsimd.tensor_scalar(
                        qt[:], R[cur][:, WB:WB + 1],
                        R[prev][:, WB:WB + 1], None, AluOpType.min,
                    )
                    # PE: pcol = shift(q), then += BIG on first-block rows
                    pcol = ps_col.tile([P, 1], F32, tag="pcol", name="pcol")
                    nc.tensor.matmul(pcol[:], sh_sb, qt[:], start=True, stop=False)
                    nc.tensor.matmul(pcol[:], bf_sb, ones1, start=False, stop=True)
                    pcols = pcols[1:] + [pcol]

            # ---- finalize: sim = 1/(1 + D/(2L)) ----
            fint = fin_pool.tile([P, 1], F32, tag="fx", name="fx")
            last = R[(NSTEPS - 1) % NR]
            nc.vector.tensor_scalar(
                fint[:], last[:, WB:WB + 1], 1.0 / (2.0 * L), 1.0,
                AluOpType.mult, AluOpType.add,
            )
            finr = fin_pool.tile([P, 1], F32, tag="fr", name="fr")
            nc.vector.reciprocal(finr[:], fint[:])
            nc.sync.dma_start(
                out_d[0:B_loc], finr[(NB - 1) * B_loc:NB * B_loc, 0:1]
            )

    nc.compile()
    return nc


def make_host_inputs(seq1, seq2, B_loc, NB, WB):
    """Per-core input dicts. seq1/seq2: (B, L, F) full arrays."""
    B, L, Fdim = seq1.shape
    P = B_loc * NB
    s12 = np.concatenate(
        [(-2.0 * seq2).transpose(0, 2, 1), seq1.transpose(0, 2, 1)], axis=2
    ).astype(np.float32)  # (B, F, 2L)

    NJB = L // 128
    sq1 = (seq1.astype(np.float64) ** 2).sum(-1).astype(np.float32)  # (B, L)
    sq2 = (seq2.astype(np.float64) ** 2).sum(-1).astype(np.float32)  # (B, L)

    # jb-major partition mapping: p = jb*B_loc + b
    cst = np.zeros((128, 259 + B_loc * NJB + 128), np.float32)
    for p in range(B_loc, P):
        cst[p - B_loc, p] = 1.0          # shift matrix
    cst[0, 128:128 + B_loc] = BIG        # bigfix row
    cst[0, 256] = 1.0                    # ones
    cst[:, 257] = BIG                    # zcol
    cst[0:B_loc, 257] = 0.0
    cst[:, 258] = BIG                    # bigcol
    cst[0, 259 + B_loc * NJB:259 + B_loc * NJB + 128] = 1.0  # ones row

    n_cores = B // B_loc
    in_maps = []
    for c in range(n_cores):
        sl = slice(c * B_loc, (c + 1) * B_loc)
        cstc = cst.copy()
        for b in range(B_loc):
            for jB in range(NJB):
                cstc[:, 259 + b * NJB + jB] = sq2[c * B_loc + b,
                                                  jB * 128:(jB + 1) * 128]
        in_maps.append({
            "s12t": np.ascontiguousarray(
                s12[sl].transpose(1, 0, 2).reshape(Fdim, B_loc * 2 * L)
            ),
            "sq1r": np.ascontiguousarray(sq1[sl].reshape(1, B_loc * L)),
            "cpack": cstc,
        })
    return in_maps


_PROGRAM_CACHE = {}


def kernel(seq1, seq2, scale_weights):
    """Full-input entry point: (32,1024,128)x2 + (1,) -> (32,) float32."""
    seq1 = np.asarray(seq1, dtype=np.float32)
    seq2 = np.asarray(seq2, dtype=np.float32)
    scale_weights = np.asarray(scale_weights, dtype=np.float32)

    B_loc = B_FULL // N_CORES
    NB, WB = 32, 32
    key = "prod"
    if key not in _PROGRAM_CACHE:
        _PROGRAM_CACHE[key] = build_program(
            B_loc=B_loc, L=L_FULL, Fdim=F_FULL, NB=NB, WB=WB
        )
    nc = _PROGRAM_CACHE[key]

    in_maps = make_host_inputs(seq1, seq2, B_loc, NB, WB)
    res = bass_utils.run_bass_kernel_spmd(
        nc, in_maps, core_ids=list(range(N_CORES))
    )
    sims = np.concatenate([r["simout"] for r in res.results]).astype(np.float32)

    # softmax over a single weight is exactly 1.0
    e = np.exp(scale_weights - scale_weights.max())
    w0 = (e / e.sum())[0].astype(np.float32)
    return (w0 * sims).astype(np.float32)



# revision 2
# speedup vs baseline: 1.0045x; 1.0045x over previous
"""DTW similarity kernel for Trainium2 (8 NeuronCores, batch-sharded).

Fused-scan version: ONE DVE tensor_tensor_scan per DP row (vs min+scan in
the baseline), using interleaved scan slots:

  pair j<W:  slotA: state = min(prevD[j-1], state) + 0
             slotB: state = min(prevD[j],   state) + c[j]   -> D[j]
  pair W:    slotA: state = min(prevD[W-1], state) + 0
                    -> q = min(D_prev[last], D[last])
             slotB: junk (bounded)

Scratch row layout (fp32, per partition, width SCRW=2W+3):
  col 0 halo(BIG) | col 1+2j junkA_j | col 2+2j D_j | col 1+2W q | col 2+2W junkB
data0 reads prev row with overlapping AP [[2,W+1],[2,2]] @ col0 (elem (j,r)
-> col 2j+2r): slotA_j sees D_{j-1}, slotB_j sees D_j, pair W slotA sees
D_{W-1} of the PREV row => q = min(D_prev[last], state=D[last]).
data1 reads the bf16 cost strip with AP [[SLOTS,W+1],[t+1,2]] @ lane0/slot0:
slotA_j hits the always-zero slot 0 of lane j, slotB_j hits lane j slot 1+t.
Lane W (extra all-zero lane) feeds pair W.

Cross-block handoff (initial = min(D[i,left-1], D[i-1,left-1]) = q of block
jb-1 from step t-SK): PE shift-matmul (batched K=2 steps) -> PSUM, ACT copy
PSUM->SBUF qshift columns; scan reads initial from SBUF (no PSUM penalty).

Per core (B_loc=4 batches), partition p = jb*B_loc + b, NB=32 blocks of
WB=32 columns, skew SK steps between adjacent blocks.
"""

import contextlib

import numpy as np

import concourse.bacc as bacc
import concourse.bass as bass
import concourse.mybir as mybir
from concourse import bass_utils
from concourse.ap import AP
from concourse.mybir import AluOpType
from concourse.tile import TileContext

F32 = mybir.dt.float32
BF16 = mybir.dt.bfloat16
BIG = 1e9

B_FULL, L_FULL, F_FULL = 32, 1024, 128
N_CORES = 8


def fused_scan(nc, out_ap, d0_ap, d1_ap, initial, op0, op1):
    eng = nc.vector
    return eng.add_instruction(
        mybir.InstTensorScalarPtr(
            name=nc.get_next_instruction_name(),
            is_tensor_tensor_scan=True,
            is_scalar_tensor_tensor=True,
            op0=op0,
            op1=op1,
            ins=[
                eng.lower_ap(d0_ap),
                eng.lower_ap_or_imm(initial),
                eng.lower_ap(d1_ap),
            ],
            outs=[eng.lower_ap(out_ap)],
        )
    )


def build_program(B_loc=4, L=1024, Fdim=128, WB=32, SK=8, NR=6, KQ=16,
                  strip_dtype=BF16, phase="all"):
    """Build the Bass program for one core handling B_loc batches."""
    NB = L // WB
    assert NB * WB == L and B_loc * NB <= 128 and Fdim <= 128
    P = B_loc * NB
    W = WB
    NSTEPS = L + SK * (NB - 1)
    SLOTS = NSTEPS + 1
    SCRW = 2 * W + 3
    NJB = L // 128            # 128-wide j-blocks (GEMM lhsT tiles)
    JPT = 128 // WB           # jb column-blocks per GEMM tile
    IW = min(L, 512)          # i-chunk (GEMM N)
    NIH = L // IW
    assert L % 128 == 0 and (L % 512 == 0 or L < 512)
    assert NR % 2 == 0 and KQ % 2 == 0 and SK < KQ - 2

    nc = bacc.Bacc(
        "TRN2", debug=False, num_devices=N_CORES,
        # Overlapping scan APs + mixed partition/offset relayout DMAs are
        # beyond the race detector's AP decomposition; Tile's dependency
        # tracking handles them (verified end-to-end vs numpy DP).
        detect_race_conditions=False,
    )

    # ---- I/O ----
    # s12: cols [0,L) = -2*seq2^T (lhsT), cols [L,2L) = seq1^T (rhs)
    s12_d = nc.dram_tensor("s12t", (Fdim, B_loc * 2 * L), BF16, kind="ExternalInput")
    # row 0: cols [0,128) ones (lhsT row for the sq1 accumulate), then sq1
    sq1_d = nc.dram_tensor("sq1r", (1, 128 + B_loc * L), BF16, kind="ExternalInput")
    # packed constants: [0,128) shift lhsT; [128,256) bigfix row (row 0);
    # 256:258 ones row (row 0); 258+b*NJB+jB sq2 cols; then 128-wide ones row
    NCST = 259 + B_loc * NJB + 128
    cst_d = nc.dram_tensor("cpack", (128, NCST), F32, kind="ExternalInput")
    out_d = nc.dram_tensor("simout", (B_loc,), F32, kind="ExternalOutput")
    # DRAM-staged cost strip skeleton (pre-zeroed by the runtime: outputs
    # are donated zero buffers), lane-major [c][slot] per partition
    stripz_d = nc.dram_tensor("stripd", (128, (WB + 1) * SLOTS), BF16,
                              kind="ExternalOutput")

    with TileContext(nc) as tc:
        with contextlib.ExitStack() as ctx:
            const_pool = ctx.enter_context(tc.tile_pool(name="const", bufs=1))
            seq_pool = ctx.enter_context(tc.tile_pool(name="seqs", bufs=1))
            stage_pool = ctx.enter_context(tc.tile_pool(name="stage", bufs=3))
            strip_pool = ctx.enter_context(tc.tile_pool(name="strip", bufs=1))
            scr_pool = ctx.enter_context(tc.tile_pool(name="scr", bufs=1))
            fin_pool = ctx.enter_context(tc.tile_pool(name="fin", bufs=1))
            ps_mm = ctx.enter_context(tc.tile_pool(name="psmm", bufs=3, space="PSUM"))
            ps_q = ctx.enter_context(tc.tile_pool(name="psq", bufs=4, space="PSUM"))

            # ---- constants / inputs to SBUF (single DMA each) ----
            cst = const_pool.tile([128, NCST], F32)
            nc.sync.dma_start(cst[:], cst_d[:, :])
            sh_sb = cst[0:128, 0:128]            # shift matrix lhsT
            bf_sb = cst[0:1, 128:256]            # bigfix lhsT row
            ones2 = cst[0:1, 256:258]            # [1,2] ones rhs

            s12_sb = seq_pool.tile([Fdim, B_loc * 2 * L], BF16)
            for _b in range(B_loc):
                nc.sync.dma_start(s12_sb[:, _b * 2 * L:(_b + 1) * 2 * L],
                                  s12_d[:, _b * 2 * L:(_b + 1) * 2 * L])
            sq1_sb = seq_pool.tile([1, 128 + B_loc * L], BF16)
            nc.sync.dma_start(sq1_sb[:], sq1_d[:, :])
            onesrow = sq1_sb[0:1, 0:128]         # bf16 ones lhsT row

            # segmented SBUF strips: seg k holds global slots
            # [k*SEGLEN, k*SEGLEN+len) at local 1.., local slot 0 = zeros
            # non-uniform: small first segments so scans start early
            seg_len = []
            rem, growth = SLOTS, [64, 128, 192]
            while rem > 0:
                ln = growth[len(seg_len)] if len(seg_len) < len(growth) else 256
                ln = min(ln, rem)
                seg_len.append(ln)
                rem -= ln
            NSEG = len(seg_len)
            seg_g0 = [sum(seg_len[:k]) for k in range(NSEG)]
            segs = [strip_pool.tile([128, (WB + 1) * (seg_len[k] + 1)],
                                    strip_dtype, name=f"seg{k}")
                    for k in range(NSEG)]
            segv = [segs[k][:].rearrange("p (c s) -> p c s", c=WB + 1)
                    for k in range(NSEG)]
            for k in range(NSEG):
                nc.gpsimd.memset(segv[k][:, :, 0:1], 0.0)
            dzv = stripz_d[:, :].rearrange("p (c s) -> p c s", c=WB + 1)

            # scratch rows, rotating NR-deep inside one tile
            scr = scr_pool.tile([128, NR * SCRW], F32)
            nc.gpsimd.memset(scr[:], BIG)
            scr_pitch = NR * SCRW

            # qshift: rotating cols 0..KQ-1; col KQ = t0 initial (0 for jb=0
            # partitions, BIG elsewhere); col KQ+1 = all-BIG (t in [1,SK))
            qshift = fin_pool.tile([128, KQ + 2], F32)
            nc.gpsimd.memset(qshift[:], BIG)
            nc.gpsimd.memset(qshift[0:B_loc, KQ:KQ + 1], 0.0)

            # ---- GEMM (C^T) + sqrt + relayout-to-strip ----
            def emit_gemm_unit(ih, jB, b):
                o = b * 2 * L
                pt = ps_mm.tile([128, IW], F32, tag="pmm", name="pmm")
                nc.tensor.matmul(
                    pt[:],
                    s12_sb[:, o + jB * 128:o + (jB + 1) * 128],
                    s12_sb[:, o + L + ih * IW:o + L + (ih + 1) * IW],
                    start=True, stop=False,
                )
                nc.tensor.matmul(
                    pt[:],
                    onesrow,
                    sq1_sb[0:1, 128 + b * L + ih * IW:
                           128 + b * L + (ih * IW + IW)],
                    start=False, stop=True,
                )
                st = stage_pool.tile([128, IW], strip_dtype,
                                     tag="cstage", name="cstage")
                # sqrt with per-partition (j) bias = sq2[j]
                sq2col = cst[0:128, 258 + b * NJB + jB:259 + b * NJB + jB]
                nc.scalar.activation(
                    st[:], pt[:], mybir.ActivationFunctionType.Sqrt,
                    bias=sq2col, scale=1.0,
                )
                # relayout: stage (128 j, IW i) -> strip lanes,
                # one DMA per jb (partition-contained views)
                jb0 = jB * JPT
                for u in range(JPT):
                    jb = jb0 + u
                    p = jb * B_loc + b
                    o1 = 1 + ih * IW + SK * jb
                    dst = dzv[p:p + 1, 0:WB, o1:o1 + IW]
                    eng = (nc.sync, nc.gpsimd, nc.scalar)[(b * JPT + u) % 3]
                    eng.dma_start(dst, st[u * WB:(u + 1) * WB, :])

            def seg_load(k):
                g0, ln = seg_g0[k], seg_len[k]
                e1 = (nc.sync, nc.gpsimd, nc.scalar)[k % 3]
                e2 = (nc.gpsimd, nc.scalar, nc.sync)[k % 3]
                e1.dma_start(segv[k][:, 0:16, 1:1 + ln],
                             dzv[:, 0:16, g0:g0 + ln])
                e2.dma_start(segv[k][:, 16:33, 1:1 + ln],
                             dzv[:, 16:33, g0:g0 + ln])

            def unit_span(ih, jB):
                lo = 1 + ih * IW + SK * jB * JPT
                return lo, lo + IW + SK * (JPT - 1)

            if phase in ("all", "gemm"):
                units = [(ih, jB, b)
                         for ih in range(NIH) for jB in range(NJB)
                         for b in range(B_loc)]
                # last unit index covering each segment
                last_unit = [0] * NSEG
                for k in range(NSEG):
                    g0, g1 = seg_g0[k], seg_g0[k] + seg_len[k]
                    for i, (ih, jB, b) in enumerate(units):
                        lo, hi = unit_span(ih, jB)
                        if lo < g1 and g0 < hi:
                            last_unit[k] = max(last_unit[k], i)
                for i, u in enumerate(units):
                    emit_gemm_unit(*u)
                    for k in range(NSEG):
                        if last_unit[k] == i:
                            seg_load(k)
            elif phase == "scan":
                # timing-only variant: segments loaded from the (zero) DRAM
                for k in range(NSEG):
                    seg_load(k)

            # ---- scan phase ----
            if phase in ("all", "scan"):
                for t in range(NSTEPS):
                    cur_o = (t % NR) * SCRW
                    prev_o = ((t - 1) % NR) * SCRW
                    b0 = scr[:, prev_o:prev_o + 1]
                    d0 = AP(b0.tensor, b0.offset,
                            [[scr_pitch, 128], [2, W + 1], [2, 2]])
                    sgl = 0
                    while seg_g0[sgl] + seg_len[sgl] <= t + 1:
                        sgl += 1
                    loc = (t + 1) - seg_g0[sgl] + 1
                    pitch_k = (WB + 1) * (seg_len[sgl] + 1)
                    b1 = segs[sgl][:, 0:1]
                    d1 = AP(b1.tensor, b1.offset,
                            [[pitch_k, 128], [seg_len[sgl] + 1, W + 1],
                             [loc, 2]])
                    bo = scr[:, cur_o + 1:cur_o + 2]
                    oap = AP(bo.tensor, bo.offset,
                             [[scr_pitch, 128], [2, W + 1], [1, 2]])
                    if t == 0:
                        ini = qshift[:, KQ:KQ + 1]
                    elif t < SK:
                        ini = qshift[:, KQ + 1:KQ + 2]
                    else:
                        c = (t - SK) % KQ
                        ini = qshift[:, c:c + 1]
                    fused_scan(nc, oap, d0, d1, ini, AluOpType.min, AluOpType.add)

                    # q handoff, batched over 2 steps (t odd): PE shift
                    # matmul of q cols (t-1, t) + bigfix, ACT copy to SBUF
                    if t % 2 == 1:
                        qo = ((t - 1) % NR) * SCRW + 1 + 2 * W
                        rq = scr[:, qo:qo + 1]
                        rhs = AP(rq.tensor, rq.offset,
                                 [[scr_pitch, 128], [SCRW, 2]])
                        psq = ps_q.tile([128, 2], F32, tag="psq", name="psq")
                        nc.tensor.matmul(psq[:], sh_sb, rhs,
                                         start=True, stop=False)
                        nc.tensor.matmul(psq[:], bf_sb, ones2,
                                         start=False, stop=True)
                        qc = (t - 1) % KQ
                        nc.scalar.copy(qshift[:, qc:qc + 2], psq[:])

            # ---- finalize: sim = 1/(1 + D/(2L)) ----
            fint = fin_pool.tile([P, 1], F32, tag="fx", name="fx")
            last_o = ((NSTEPS - 1) % NR) * SCRW
            fcol = scr[:, last_o + 2 * W:last_o + 2 * W + 1]
            nc.vector.tensor_scalar(
                fint[:], fcol[0:P, 0:1], 1.0 / (2.0 * L), 1.0,
                AluOpType.mult, AluOpType.add,
            )
            finr = fin_pool.tile([P, 1], F32, tag="fr", name="fr")
            nc.vector.reciprocal(finr[:], fint[:])
            nc.sync.dma_start(
                out_d[0:B_loc], finr[(NB - 1) * B_loc:NB * B_loc, 0:1]
            )

    nc.compile()
    return nc


def make_host_inputs(seq1, seq2, B_loc, NB, WB):
    """Per-core input dicts. seq1/seq2: (B, L, F) full arrays."""
    B, L, Fdim = seq1.shape
    P = B_loc * NB
    s12 = np.concatenate(
        [(-2.0 * seq2).transpose(0, 2, 1), seq1.transpose(0, 2, 1)], axis=2
    ).astype(np.float32)  # (B, F, 2L)

    NJB = L // 128
    sq1 = (seq1.astype(np.float64) ** 2).sum(-1).astype(np.float32)  # (B, L)
    sq2 = (seq2.astype(np.float64) ** 2).sum(-1).astype(np.float32)  # (B, L)

    NCST = 259 + B_loc * NJB + 128
    cst = np.zeros((128, NCST), np.float32)
    for p in range(B_loc, P):
        cst[p - B_loc, p] = 1.0          # shift matrix
    cst[0, 128:128 + B_loc] = BIG        # bigfix row
    cst[0, 256:258] = 1.0                # ones2
    cst[0, 258 + B_loc * NJB:258 + B_loc * NJB + 128] = 1.0  # ones row
    cst[0:B_loc, 258 + B_loc * NJB + 128] = BIG  # bigfix bias col

    n_cores = B // B_loc
    in_maps = []
    for c in range(n_cores):
        sl = slice(c * B_loc, (c + 1) * B_loc)
        cstc = cst.copy()
        for b in range(B_loc):
            for jB in range(NJB):
                cstc[:, 258 + b * NJB + jB] = sq2[c * B_loc + b,
                                                  jB * 128:(jB + 1) * 128]
        import jax.numpy as jnp
        s12c = np.ascontiguousarray(
            s12[sl].transpose(1, 0, 2).reshape(Fdim, B_loc * 2 * L))
        sq1c = np.concatenate(
            [np.ones((1, 128), np.float32), sq1[sl].reshape(1, B_loc * L)],
            axis=1)
        in_maps.append({
            "s12t": np.asarray(jnp.asarray(s12c, dtype=jnp.bfloat16)),
            "sq1r": np.asarray(jnp.asarray(sq1c, dtype=jnp.bfloat16)),
            "cpack": cstc,
        })
    return in_maps


_PROGRAM_CACHE = {}


def kernel(seq1, seq2, scale_weights):
    """Full-input entry point: (32,1024,128)x2 + (1,) -> (32,) float32."""
    seq1 = np.asarray(seq1, dtype=np.float32)
    seq2 = np.asarray(seq2, dtype=np.float32)
    scale_weights = np.asarray(scale_weights, dtype=np.float32)

    B_loc = B_FULL // N_CORES
    NB, WB = 32, 32
    key = "prod"
    if key not in _PROGRAM_CACHE:
        _PROGRAM_CACHE[key] = build_program(
            B_loc=B_loc, L=L_FULL, Fdim=F_FULL, WB=WB
        )
    nc = _PROGRAM_CACHE[key]

    in_maps = make_host_inputs(seq1, seq2, B_loc, NB, WB)
    res = bass_utils.run_bass_kernel_spmd(
        nc, in_maps, core_ids=list(range(N_CORES))
    )
    sims = np.concatenate([r["simout"] for r in res.results]).astype(np.float32)

    # softmax over a single weight is exactly 1.0
    e = np.exp(scale_weights - scale_weights.max())
    w0 = (e / e.sum())[0].astype(np.float32)
    return (w0 * sims).astype(np.float32)
